# revision 32
# baseline (speedup 1.0000x reference)
"""Trainium2 Bass kernel for nn_Attention_49907519980190.

Reference computation (b=2, n=2048, dim=1024, h=16, d=64):
    q = (x @ w_q)   -> (b, h, n, d)
    k, v = split(x @ w_vk)
    dots = (q @ k^T) * sqrt(d)          # NOTE: multiplies by 8
    attn = softmax(dots)
    out = (attn @ v) reassembled -> (b, n, h*d) @ w_out

Sharding (8 cores): batch x head-group parallel. Core c handles batch
b = c // 4 and heads 4*(c % 4) .. 4*(c % 4) + 4. Column-parallel
q/k/v projections, row-parallel out projection; the host sums the four
partial outputs per batch (the "all-reduce" of row-parallel TP).

Numerics: the softmax logits have std ~75 and the softmax is ~97%
one-hot, so Q/K/dots need fp32-class precision. The PE's in-matmul
accumulator is block-aligned (drops addends ~2^-11 below the running
max) but PSUM accumulation BETWEEN matmuls is exact fp32. So Q, K and
dots use a bf16 hi/lo decomposition: x = hi + lo with both bf16;
a@b ~= ah@bh + (al@bh + ah@bl), a main matmul plus one stacked
correction matmul ([qh;ql] x [kl;kh]) accumulating in PSUM
(measured ~5e-6 matmul rel err vs 2.4e-3 plain bf16).

Softmax: instead of an exact row max (expensive full-width DVE reduce
of PSUM), use a stride-2 subset max m^ and compute
u' = exp((l - 8*m^)/2) in fp32, then u = u'^2 on GPSIMD (exp-squared
doubles the usable logit-gap range to ~176; P(subset max short by
>176) ~ 1e-8 per row). The softmax denominator comes for free as a
65th all-ones column appended per-head to V (the PV matmul then
produces sum_j u_ij in column 64). Attention probabilities and the
whole post-softmax path are fp16.

Schedule: software-pipelined attention loop (PV for iteration N-3
issues between the dots of iteration N), PSUM S-quarters recycle
progressively, projections start as soon as the first token-group of
x^T lands, DMA-transposes all stay on the sync queue (issuing
transposes from two queues concurrently corrupts data - measured).
"""

import numpy as np

import concourse.bass as bass
import concourse.mybir as mybir
import concourse.tile as tile
from concourse import bacc
from concourse.bass_utils import run_bass_kernel_spmd
from concourse.masks import make_identity

F32 = mybir.dt.float32
BF16 = mybir.dt.bfloat16
FP16 = mybir.dt.float16
SUB = mybir.AluOpType.subtract
MULT = mybir.AluOpType.mult
MAX = mybir.AluOpType.max
AX = mybir.AxisListType.X
EXP = mybir.ActivationFunctionType.Exp

P = 128      # partitions
NTOK = 2048  # tokens per core (one batch slice)
DIM = 1024   # model dim
E = 256      # per-core projection width (4 heads x 64)
NH = 4       # heads per core
D = 64       # head dim
D1 = 65      # head dim + ones column (denominator trick)
KO = 8       # contraction chunks of 128 over DIM
TT = 16      # token tiles of 128
NG = 4       # token groups (of 512) for x^T
SCALE = 8.0  # sqrt(D); reference MULTIPLIES by it


def build_attention_nc():
    nc = bacc.Bacc("TRN2", target_bir_lowering=False, debug=False)

    x = nc.declare_dram_parameter("x", [NTOK, DIM], F32, isOutput=False)
    wq = nc.declare_dram_parameter("wq", [DIM, E], F32, isOutput=False)
    wk = nc.declare_dram_parameter("wk", [DIM, E], F32, isOutput=False)
    wv = nc.declare_dram_parameter("wv", [DIM, E], F32, isOutput=False)
    wo = nc.declare_dram_parameter("wo", [E, DIM], F32, isOutput=False)
    y = nc.declare_dram_parameter("y", [NTOK, DIM], F32, isOutput=True)

    with tile.TileContext(nc) as tc:
        with tc.tile_pool(name="persist", bufs=1) as persist:
            # Q^T stacked per head: rows 0:64 = q_hi, 64:128 = q_lo
            QTs = persist.tile([P, NH, NTOK], BF16)
            # K^T swapped-stacked: rows 0:64 = k_lo, 64:128 = k_hi
            KTsw = persist.tile([P, NH, NTOK], BF16)
            # K^T hi over zeros: main dots matmul shares the full 128-row
            # stationary [q_hi; q_lo] with the correction matmul (rows 64:128
            # multiply q_lo by zero)
            KThz = persist.tile([P, NH, NTOK], BF16)
            # V natural [tok_low, tok_hi, head*(64+ones)] fp16
            Vb = persist.tile([P, TT, NH * D1], FP16)
            Ob = persist.tile([P, TT, E], FP16)
            # O^T [emb_low, emb_hi(2), tok]
            OTb = persist.tile([P, 2, NTOK], FP16)
            wob = persist.tile([P, 2, DIM], FP16)
            ident = persist.tile([P, P], FP16)
            make_identity(nc, ident)
            identF = persist.tile([P, P], F32)
            make_identity(nc, identF)
            # ones columns of Vb (written once)
            vb4 = Vb.rearrange("p t (h c) -> p t h c", c=D1)
            nc.vector.memset(vb4[:, :, :, D:D1], 1.0)
            nc.vector.memset(KThz[64:128, :, :], 0.0)

            with tc.tile_pool(name="xpool", bufs=1) as xpool:
                # x^T hi/lo bf16 in 4 token-group tiles: [dim_low, dim_hi, 512]
                xh = []
                xl = []
                for g in range(NG):
                    xh_g = xpool.tile([P, KO, 512], BF16, tag=f"xh{g}")
                    xl_g = xpool.tile([P, KO, 512], BF16, tag=f"xl{g}")
                    xh.append(xh_g)
                    xl.append(xl_g)
                wqh = xpool.tile([P, KO, E], BF16)
                wql = xpool.tile([P, KO, E], BF16)
                wkh = xpool.tile([P, KO, E], BF16)
                wkl = xpool.tile([P, KO, E], BF16)
                wvh = xpool.tile([P, KO, E], BF16)

                # ---------- Phase A/B: weights, x split hi/lo + transpose
                with (
                    tc.tile_pool(name="psT", bufs=3, space="PSUM") as psT,
                    tc.tile_pool(name="psA", bufs=2, space="PSUM") as psA,
                    tc.tile_pool(name="stage", bufs=3) as stage,
                ):
                    for wsrc, hdst, ldst in ((wk, wkh, wkl), (wq, wqh, wql)):
                        wf = stage.tile([P, KO, E], F32, tag="wf", bufs=1)
                        nc.sync.dma_start(
                            out=wf,
                            in_=wsrc[:, :].rearrange("(ko p) e -> p ko e", p=P),
                        )
                        nc.vector.tensor_copy(out=hdst, in_=wf)
                        nc.vector.tensor_tensor(out=ldst, in0=wf, in1=hdst, op=SUB)
                    wf = stage.tile([P, KO, E], F32, tag="wf", bufs=1)
                    nc.sync.dma_start(
                        out=wf, in_=wv[:, :].rearrange("(ko p) e -> p ko e", p=P)
                    )
                    nc.scalar.copy(out=wvh, in_=wf)
                    wof = stage.tile([P, 2, DIM], F32, tag="wof", bufs=1)
                    nc.sync.dma_start(
                        out=wof, in_=wo[:, :].rearrange("(eo p) d -> p eo d", p=P)
                    )
                    nc.scalar.copy(out=wob, in_=wof)

                    for tt in range(TT):
                        g, lt = tt // 4, tt % 4
                        ts = slice(tt * P, (tt + 1) * P)
                        gs = slice(lt * P, (lt + 1) * P)
                        xf = stage.tile([P, DIM], F32, tag="xf")
                        nc.gpsimd.dma_start(out=xf, in_=x[ts, :])
                        for half in range(2):
                            cr = slice(half * 4, half * 4 + 4)
                            pt4 = psT.tile([P, 4, P], F32, tag="pt4")
                            for c4 in range(4):
                                c = half * 4 + c4
                                nc.tensor.transpose(
                                    pt4[:, c4, :], xf[:, c * P:(c + 1) * P],
                                    identF[:, :],
                                )
                            nc.vector.tensor_copy(
                                out=xh[g][:, cr, gs], in_=pt4
                            )
                            nc.vector.tensor_tensor(
                                out=xl[g][:, cr, gs], in0=pt4,
                                in1=xh[g][:, cr, gs], op=SUB,
                            )

                # ---------- Phase C: projections K^T, Q^T (hi/lo), V (plain)
                if True:
                    def proj_hilo(wh, wl, m, g):
                        pr = psA.tile([P, 512], F32, tag="pr")
                        ms = slice(m * P, (m + 1) * P)
                        for c in range(KO):
                            nc.tensor.matmul(
                                pr[:, :], wh[:, c, ms], xh[g][:, c, :],
                                start=(c == 0), stop=False,
                            )
                        for c in range(KO):
                            nc.tensor.matmul(
                                pr[:, :], wl[:, c, ms], xh[g][:, c, :],
                                start=False, stop=False,
                            )
                        for c in range(KO):
                            nc.tensor.matmul(
                                pr[:, :], wh[:, c, ms], xl[g][:, c, :],
                                start=False, stop=(c == KO - 1),
                            )
                        return pr

                    for g in range(NG):
                        ns = slice(g * 512, (g + 1) * 512)
                        for m in range(2):
                            pr = proj_hilo(wkh, wkl, m, g)
                            for hh in range(2):
                                h = 2 * m + hh
                                rows = slice(hh * 64, hh * 64 + 64)
                                nc.scalar.copy(
                                    out=KThz[0:64, h, ns], in_=pr[rows, :]
                                )
                                nc.scalar.copy(
                                    out=KTsw[64:128, h, ns], in_=pr[rows, :]
                                )
                                nc.vector.tensor_tensor(
                                    out=KTsw[0:64, h, ns], in0=pr[rows, :],
                                    in1=KThz[0:64, h, ns], op=SUB,
                                )
                        for m in range(2):
                            pr = proj_hilo(wqh, wql, m, g)
                            for hh in range(2):
                                h = 2 * m + hh
                                rows = slice(hh * 64, hh * 64 + 64)
                                nc.scalar.copy(
                                    out=QTs[0:64, h, ns], in_=pr[rows, :]
                                )
                                nc.vector.tensor_tensor(
                                    out=QTs[64:128, h, ns], in0=pr[rows, :],
                                    in1=QTs[0:64, h, ns], op=SUB,
                                )
                        for lt in range(4):
                            tm = g * 4 + lt
                            pr = psA.tile([P, 512], F32, tag="pr")
                            for c in range(KO):
                                nc.tensor.matmul(
                                    pr[:, :E], xh[g][:, c, lt * P:(lt + 1) * P],
                                    wvh[:, c, :],
                                    start=(c == 0), stop=(c == KO - 1),
                                )
                            for h in range(NH):
                                nc.scalar.copy(
                                    out=Vb[:, tm, h * D1:h * D1 + D],
                                    in_=pr[:, h * D:(h + 1) * D],
                                )

            # ---------- Phase D: attention, software-pipelined depth 3
            with (
                tc.tile_pool(name="psO", bufs=1, space="PSUM") as psO,
                tc.tile_pool(name="attn_sb", bufs=3) as attn_sb,
                tc.tile_pool(name="attn_small", bufs=6) as attn_small,
                tc.tile_pool(name="ysb", bufs=3) as ysb,
            ):
                pending = []

                def issue_pv(h, it, PT):
                    O_ps = psO.tile([P, D1], F32, tag="O")
                    for jo in range(TT):
                        nc.tensor.matmul(
                            O_ps[:, :], PT[:, jo, :],
                            Vb[:, jo, h * D1:(h + 1) * D1],
                            start=(jo == 0), stop=(jo == TT - 1),
                        )
                    rec = attn_small.tile([P, 1], F32, tag="rec")
                    nc.vector.reciprocal(out=rec, in_=O_ps[:, D:D1])
                    nc.scalar.activation(
                        out=Ob[:, it, h * D:(h + 1) * D], in_=O_ps[:, :D],
                        func=mybir.ActivationFunctionType.Copy, scale=rec,
                    )

                with tc.tile_pool(name="psS", bufs=7, space="PSUM") as psS:
                    def issue_dots(h, it):
                        isl = slice(it * P, (it + 1) * P)
                        quarters = []
                        for nn in range(4):
                            Sq = psS.tile([P, 512], F32, tag="S")
                            quarters.append(Sq)
                        # all 8 matmuls share one stationary [q_hi; q_lo]
                        for nn in range(4):
                            ns = slice(nn * 512, (nn + 1) * 512)
                            nc.tensor.matmul(
                                quarters[nn][:, :], QTs[:, h, isl], KThz[:, h, ns],
                                start=True, stop=False,
                            )
                        for nn in range(4):
                            ns = slice(nn * 512, (nn + 1) * 512)
                            nc.tensor.matmul(
                                quarters[nn][:, :], QTs[:, h, isl], KTsw[:, h, ns],
                                start=False, stop=True,
                            )
                        return quarters

                    def issue_softmax(h, it, quarters):
                        mx4 = attn_small.tile([P, 4], F32, tag="mx4")
                        for nn in range(4):
                            nc.vector.tensor_reduce(
                                out=mx4[:, nn:nn + 1], in_=quarters[nn],
                                axis=AX, op=MAX,
                            )
                        nmx = attn_small.tile([P, 1], F32, tag="nmx")
                        nc.vector.tensor_reduce(
                            out=nmx, in_=mx4, axis=AX, op=MAX, negate=True
                        )
                        bias8 = attn_small.tile([P, 1], F32, tag="bias8")
                        nc.gpsimd.tensor_scalar_mul(bias8, nmx, SCALE)
                        u = attn_sb.tile([P, NTOK], FP16, tag="u", bufs=3)
                        for nn in range(4):
                            cs = slice(nn * 512, (nn + 1) * 512)
                            nc.scalar.activation(
                                out=u[:, cs], in_=quarters[nn], func=EXP,
                                bias=bias8, scale=SCALE,
                            )
                        PT = attn_sb.tile([P, TT, P], FP16, tag="PT", bufs=7)
                        nc.sync.dma_start_transpose(out=PT, in_=u)
                        return PT

                    step = 0
                    for h in range(NH):
                        for it in range(TT):
                            quarters = issue_dots(h, it)
                            # batch PV issue in pairs every other iteration so
                            # the dots blocks form long dense PE stretches
                            if step % 2 == 1:
                                while len(pending) > 4:
                                    issue_pv(*pending.pop(0))
                            PT = issue_softmax(h, it, quarters)
                            pending.append((h, it, PT))
                            step += 1

                # ---------- Phase E interleaved with the PV drain
                # (the S banks are freed; psE reuses them)
                with tc.tile_pool(name="psE", bufs=2, space="PSUM") as psE:
                    def issue_E(tm):
                        ts2 = slice(tm * P, (tm + 1) * P)
                        for eo in range(2):
                            pt = psE.tile([P, P], FP16, tag="pt")
                            nc.tensor.transpose(
                                pt[:, :], Ob[:, tm, eo * P:(eo + 1) * P],
                                ident[:, :],
                            )
                            nc.vector.tensor_copy(out=OTb[:, eo, ts2], in_=pt)
                        for n in range(2):
                            ns2 = slice(n * 512, (n + 1) * 512)
                            yp = psE.tile([P, 512], F32, tag="yp")
                            for eo in range(2):
                                nc.tensor.matmul(
                                    yp[:, :], OTb[:, eo, ts2], wob[:, eo, ns2],
                                    start=(eo == 0), stop=(eo == 1),
                                )
                            yo = ysb.tile([P, 512], F32, tag="yo")
                            nc.vector.tensor_copy(out=yo, in_=yp)
                            eng = nc.sync if (tm + n) % 2 == 0 else nc.scalar
                            eng.dma_start(out=y[ts2, ns2], in_=yo)

                    nextE = 0
                    while pending:
                        issue_pv(*pending.pop(0))
                        ready = TT - len(pending)
                        for _ in range(3):
                            if nextE < ready - 1:
                                issue_E(nextE)
                                nextE += 1
                    while nextE < TT:
                        issue_E(nextE)
                        nextE += 1

    nc.compile()
    return nc


_NC_CACHE = None


def _get_nc():
    global _NC_CACHE
    if _NC_CACHE is None:
        _NC_CACHE = build_attention_nc()
    return _NC_CACHE


def kernel(x, w_q, w_vk, w_out, **run_kwargs):
    """Full inputs in, full output out. Shards over 8 NeuronCores."""
    b, n, dim = x.shape
    assert (b, n, dim) == (2, 2048, 1024)
    w_k = w_vk[:, :1024]
    w_v = w_vk[:, 1024:]

    in_maps = []
    for c in range(8):
        bi = c // 4
        hg = c % 4
        cs = slice(hg * E, (hg + 1) * E)
        in_maps.append({
            "x": np.ascontiguousarray(x[bi]).astype(np.float32),
            "wq": np.ascontiguousarray(w_q[:, cs]).astype(np.float32),
            "wk": np.ascontiguousarray(w_k[:, cs]).astype(np.float32),
            "wv": np.ascontiguousarray(w_v[:, cs]).astype(np.float32),
            "wo": np.ascontiguousarray(w_out[cs, :]).astype(np.float32),
        })

    nc = _get_nc()
    res = run_bass_kernel_spmd(nc, in_maps, core_ids=list(range(8)), **run_kwargs)
    out = np.zeros((2, 2048, 1024), dtype=np.float32)
    for c in range(8):
        out[c // 4] += res.results[c]["y"]
    if run_kwargs:
        kernel.last_results = res
    return out


# revision 33
# speedup vs baseline: 1.1751x; 1.1751x over previous
"""Trainium2 Bass kernel for nn_Attention_49907519980190.

Reference computation (b=2, n=2048, dim=1024, h=16, d=64):
    q = (x @ w_q)   -> (b, h, n, d)
    k, v = split(x @ w_vk)
    dots = (q @ k^T) * sqrt(d)          # NOTE: multiplies by 8
    attn = softmax(dots)
    out = (attn @ v) reassembled -> (b, n, h*d) @ w_out

Sharding (8 cores): batch x head-group parallel. Core c handles batch
b = c // 4 and heads 4*(c % 4) .. 4*(c % 4) + 4. Column-parallel
q/k/v projections, row-parallel out projection; the host sums the four
partial outputs per batch (the "all-reduce" of row-parallel TP).

Numerics: the softmax logits have std ~75 and the softmax is ~97%
one-hot, so Q/K/dots need fp32-class precision. The PE's in-matmul
accumulator is block-aligned (drops addends ~2^-11 below the running
max) but PSUM accumulation BETWEEN matmuls is exact fp32. So Q, K and
dots use a bf16 hi/lo decomposition: x = hi + lo with both bf16;
a@b ~= ah@bh + (al@bh + ah@bl), a main matmul plus one stacked
correction matmul ([qh;ql] x [kl;kh]) accumulating in PSUM
(measured ~5e-6 matmul rel err vs 2.4e-3 plain bf16).

Softmax: instead of an exact row max (expensive full-width DVE reduce
of PSUM), use a stride-2 subset max m^ and compute
u' = exp((l - 8*m^)/2) in fp32, then u = u'^2 on GPSIMD (exp-squared
doubles the usable logit-gap range to ~176; P(subset max short by
>176) ~ 1e-8 per row). The softmax denominator comes for free as a
65th all-ones column appended per-head to V (the PV matmul then
produces sum_j u_ij in column 64). Attention probabilities and the
whole post-softmax path are fp16.

Schedule: software-pipelined attention loop (PV for iteration N-3
issues between the dots of iteration N), PSUM S-quarters recycle
progressively, projections start as soon as the first token-group of
x^T lands, DMA-transposes all stay on the sync queue (issuing
transposes from two queues concurrently corrupts data - measured).
"""

import numpy as np

import concourse.bass as bass
import concourse.mybir as mybir
import concourse.tile as tile
from concourse import bacc
from concourse.bass_utils import run_bass_kernel_spmd
from concourse.masks import make_identity

F32 = mybir.dt.float32
BF16 = mybir.dt.bfloat16
FP16 = mybir.dt.float16
SUB = mybir.AluOpType.subtract
MULT = mybir.AluOpType.mult
MAX = mybir.AluOpType.max
AX = mybir.AxisListType.X
EXP = mybir.ActivationFunctionType.Exp

P = 128      # partitions
NTOK = 2048  # tokens per core (one batch slice)
DIM = 1024   # model dim
E = 256      # per-core projection width (4 heads x 64)
NH = 4       # heads per core
D = 64       # head dim
D1 = 65      # head dim + ones column (denominator trick)
KO = 8       # contraction chunks of 128 over DIM
TT = 16      # token tiles of 128
NG = 4       # token groups (of 512) for x^T
SCALE = 8.0  # sqrt(D); reference MULTIPLIES by it


def build_attention_nc():
    nc = bacc.Bacc("TRN2", target_bir_lowering=False, debug=False)

    x = nc.declare_dram_parameter("x", [NTOK, DIM], F32, isOutput=False)
    wq = nc.declare_dram_parameter("wq", [DIM, E], F32, isOutput=False)
    wk = nc.declare_dram_parameter("wk", [DIM, E], F32, isOutput=False)
    wv = nc.declare_dram_parameter("wv", [DIM, E], F32, isOutput=False)
    wo = nc.declare_dram_parameter("wo", [E, DIM], F32, isOutput=False)
    y = nc.declare_dram_parameter("y", [NTOK, DIM], F32, isOutput=True)

    with tile.TileContext(nc) as tc:
        with tc.tile_pool(name="persist", bufs=1) as persist:
            # Q^T stacked per head: rows 0:64 = q_hi, 64:128 = q_lo
            QTs = persist.tile([P, NH, NTOK], BF16)
            # K^T swapped-stacked: rows 0:64 = k_lo, 64:128 = k_hi
            KTsw = persist.tile([P, NH, NTOK], BF16)
            # K^T hi over zeros: main dots matmul shares the full 128-row
            # stationary [q_hi; q_lo] with the correction matmul (rows 64:128
            # multiply q_lo by zero)
            KThz = persist.tile([P, NH, NTOK], BF16)
            # V natural [tok_low, tok_hi, head*(64+ones)] fp16
            Vb = persist.tile([P, TT, NH * D1], FP16)
            Ob = persist.tile([P, TT, E], FP16)
            # O^T [emb_low, emb_hi(2), tok]
            OTb = persist.tile([P, 2, NTOK], FP16)
            wob = persist.tile([P, 2, DIM], FP16)
            ident = persist.tile([P, P], FP16)
            make_identity(nc, ident)
            identF = persist.tile([P, P], F32)
            make_identity(nc, identF)
            # ones columns of Vb (written once)
            vb4 = Vb.rearrange("p t (h c) -> p t h c", c=D1)
            nc.vector.memset(vb4[:, :, :, D:D1], 1.0)
            nc.vector.memset(KThz[64:128, :, :], 0.0)

            with tc.tile_pool(name="xpool", bufs=1) as xpool:
                # x^T hi/lo bf16 in 4 token-group tiles: [dim_low, dim_hi, 512]
                xh = []
                xl = []
                for g in range(NG):
                    xh_g = xpool.tile([P, KO, 512], BF16, tag=f"xh{g}")
                    xl_g = xpool.tile([P, KO, 512], BF16, tag=f"xl{g}")
                    xh.append(xh_g)
                    xl.append(xl_g)
                wqh = xpool.tile([P, KO, E], BF16)
                wql = xpool.tile([P, KO, E], BF16)
                wkh = xpool.tile([P, KO, E], BF16)
                wkl = xpool.tile([P, KO, E], BF16)
                wvh = xpool.tile([P, KO, E], BF16)

                # ---------- Phase A/B: weights, x split hi/lo + transpose
                with (
                    tc.tile_pool(name="psT", bufs=3, space="PSUM") as psT,
                    tc.tile_pool(name="psA", bufs=2, space="PSUM") as psA,
                    tc.tile_pool(name="stage", bufs=3) as stage,
                ):
                    for wsrc, hdst, ldst in ((wk, wkh, wkl), (wq, wqh, wql)):
                        wf = stage.tile([P, KO, E], F32, tag="wf", bufs=1)
                        nc.sync.dma_start(
                            out=wf,
                            in_=wsrc[:, :].rearrange("(ko p) e -> p ko e", p=P),
                        )
                        nc.vector.tensor_copy(out=hdst, in_=wf)
                        nc.vector.tensor_tensor(out=ldst, in0=wf, in1=hdst, op=SUB)
                    wf = stage.tile([P, KO, E], F32, tag="wf", bufs=1)
                    nc.sync.dma_start(
                        out=wf, in_=wv[:, :].rearrange("(ko p) e -> p ko e", p=P)
                    )
                    nc.scalar.copy(out=wvh, in_=wf)
                    wof = stage.tile([P, 2, DIM], F32, tag="wof", bufs=1)
                    nc.sync.dma_start(
                        out=wof, in_=wo[:, :].rearrange("(eo p) d -> p eo d", p=P)
                    )
                    nc.scalar.copy(out=wob, in_=wof)

                    for tt in range(TT):
                        g, lt = tt // 4, tt % 4
                        ts = slice(tt * P, (tt + 1) * P)
                        gs = slice(lt * P, (lt + 1) * P)
                        xf = stage.tile([P, DIM], F32, tag="xf")
                        nc.gpsimd.dma_start(out=xf, in_=x[ts, :])
                        for half in range(2):
                            cr = slice(half * 4, half * 4 + 4)
                            pt4 = psT.tile([P, 4, P], F32, tag="pt4")
                            for c4 in range(4):
                                c = half * 4 + c4
                                nc.tensor.transpose(
                                    pt4[:, c4, :], xf[:, c * P:(c + 1) * P],
                                    identF[:, :],
                                )
                            nc.vector.tensor_copy(
                                out=xh[g][:, cr, gs], in_=pt4
                            )
                            nc.vector.tensor_tensor(
                                out=xl[g][:, cr, gs], in0=pt4,
                                in1=xh[g][:, cr, gs], op=SUB,
                            )

                # ---------- Phase C: projections K^T, Q^T (hi/lo), V (plain)
                if True:
                    def proj_hilo(wh, wl, m, g):
                        pr = psA.tile([P, 512], F32, tag="pr")
                        ms = slice(m * P, (m + 1) * P)
                        for c in range(KO):
                            nc.tensor.matmul(
                                pr[:, :], wh[:, c, ms], xh[g][:, c, :],
                                start=(c == 0), stop=False,
                            )
                        for c in range(KO):
                            nc.tensor.matmul(
                                pr[:, :], wl[:, c, ms], xh[g][:, c, :],
                                start=False, stop=False,
                            )
                        for c in range(KO):
                            nc.tensor.matmul(
                                pr[:, :], wh[:, c, ms], xl[g][:, c, :],
                                start=False, stop=(c == KO - 1),
                            )
                        return pr

                    for g in range(NG):
                        ns = slice(g * 512, (g + 1) * 512)
                        for m in range(2):
                            pr = proj_hilo(wkh, wkl, m, g)
                            for hh in range(2):
                                h = 2 * m + hh
                                rows = slice(hh * 64, hh * 64 + 64)
                                nc.scalar.copy(
                                    out=KThz[0:64, h, ns], in_=pr[rows, :]
                                )
                                nc.scalar.copy(
                                    out=KTsw[64:128, h, ns], in_=pr[rows, :]
                                )
                                nc.vector.tensor_tensor(
                                    out=KTsw[0:64, h, ns], in0=pr[rows, :],
                                    in1=KThz[0:64, h, ns], op=SUB,
                                )
                        for m in range(2):
                            pr = proj_hilo(wqh, wql, m, g)
                            for hh in range(2):
                                h = 2 * m + hh
                                rows = slice(hh * 64, hh * 64 + 64)
                                nc.scalar.copy(
                                    out=QTs[0:64, h, ns], in_=pr[rows, :]
                                )
                                nc.vector.tensor_tensor(
                                    out=QTs[64:128, h, ns], in0=pr[rows, :],
                                    in1=QTs[0:64, h, ns], op=SUB,
                                )
                        for lt in range(4):
                            tm = g * 4 + lt
                            pr = psA.tile([P, 512], F32, tag="pr")
                            for c in range(KO):
                                nc.tensor.matmul(
                                    pr[:, :E], xh[g][:, c, lt * P:(lt + 1) * P],
                                    wvh[:, c, :],
                                    start=(c == 0), stop=(c == KO - 1),
                                )
                            for h in range(NH):
                                nc.scalar.copy(
                                    out=Vb[:, tm, h * D1:h * D1 + D],
                                    in_=pr[:, h * D:(h + 1) * D],
                                )

            # ---------- Phase D: attention, software-pipelined depth 3
            with (
                tc.tile_pool(name="psS", bufs=7, space="PSUM") as psS,
                tc.tile_pool(name="psO", bufs=1, space="PSUM") as psO,
                tc.tile_pool(name="attn_sb", bufs=3) as attn_sb,
                tc.tile_pool(name="attn_small", bufs=6) as attn_small,
            ):
                pending = []

                def issue_dots(h, it):
                    isl = slice(it * P, (it + 1) * P)
                    quarters = []
                    for nn in range(4):
                        Sq = psS.tile([P, 512], F32, tag="S")
                        quarters.append(Sq)
                    # all 8 matmuls share one stationary [q_hi; q_lo]
                    for nn in range(4):
                        ns = slice(nn * 512, (nn + 1) * 512)
                        nc.tensor.matmul(
                            quarters[nn][:, :], QTs[:, h, isl], KThz[:, h, ns],
                            start=True, stop=False,
                        )
                    for nn in range(4):
                        ns = slice(nn * 512, (nn + 1) * 512)
                        nc.tensor.matmul(
                            quarters[nn][:, :], QTs[:, h, isl], KTsw[:, h, ns],
                            start=False, stop=True,
                        )
                    return quarters

                def issue_softmax(h, it, quarters):
                    mx4 = attn_small.tile([P, 4], F32, tag="mx4")
                    for nn in range(4):
                        nc.vector.tensor_reduce(
                            out=mx4[:, nn:nn + 1], in_=quarters[nn], axis=AX, op=MAX
                        )
                    nmx = attn_small.tile([P, 1], F32, tag="nmx")
                    nc.vector.tensor_reduce(
                        out=nmx, in_=mx4, axis=AX, op=MAX, negate=True
                    )
                    bias8 = attn_small.tile([P, 1], F32, tag="bias8")
                    nc.gpsimd.tensor_scalar_mul(bias8, nmx, SCALE)
                    u = attn_sb.tile([P, NTOK], FP16, tag="u", bufs=4)
                    for nn in range(4):
                        cs = slice(nn * 512, (nn + 1) * 512)
                        nc.scalar.activation(
                            out=u[:, cs], in_=quarters[nn], func=EXP,
                            bias=bias8, scale=SCALE,
                        )
                    PT = attn_sb.tile([P, TT, P], FP16, tag="PT", bufs=7)
                    nc.sync.dma_start_transpose(out=PT, in_=u)
                    return PT

                def issue_pv(h, it, PT):
                    O_ps = psO.tile([P, D1], F32, tag="O")
                    for jo in range(TT):
                        nc.tensor.matmul(
                            O_ps[:, :], PT[:, jo, :],
                            Vb[:, jo, h * D1:(h + 1) * D1],
                            start=(jo == 0), stop=(jo == TT - 1),
                        )
                    rec = attn_small.tile([P, 1], F32, tag="rec")
                    nc.vector.reciprocal(out=rec, in_=O_ps[:, D:D1])
                    nc.scalar.activation(
                        out=Ob[:, it, h * D:(h + 1) * D], in_=O_ps[:, :D],
                        func=mybir.ActivationFunctionType.Copy, scale=rec,
                    )

                step = 0
                for h in range(NH):
                    for it in range(TT):
                        quarters = issue_dots(h, it)
                        # batch PV issue in pairs every other iteration so the
                        # dots blocks form long dense PE-array stretches (HAM)
                        if step % 2 == 1:
                            while len(pending) > 4:
                                issue_pv(*pending.pop(0))
                        PT = issue_softmax(h, it, quarters)
                        pending.append((h, it, PT))
                        step += 1
                while pending:
                    issue_pv(*pending.pop(0))

            # ---------- Phase E: O^T then y = O @ wo
            with (
                tc.tile_pool(name="psE", bufs=2, space="PSUM") as psE,
                tc.tile_pool(name="ysb", bufs=3) as ysb,
            ):
                for tt in range(TT):
                    ts = slice(tt * P, (tt + 1) * P)
                    for eo in range(2):
                        pt = psE.tile([P, P], FP16, tag="pt")
                        nc.tensor.transpose(
                            pt[:, :], Ob[:, tt, eo * P:(eo + 1) * P], ident[:, :]
                        )
                        nc.vector.tensor_copy(out=OTb[:, eo, ts], in_=pt)
                for tm in range(TT):
                    ms = slice(tm * P, (tm + 1) * P)
                    for n in range(2):
                        ns = slice(n * 512, (n + 1) * 512)
                        yp = psE.tile([P, 512], F32, tag="yp")
                        for eo in range(2):
                            nc.tensor.matmul(
                                yp[:, :], OTb[:, eo, ms], wob[:, eo, ns],
                                start=(eo == 0), stop=(eo == 1),
                            )
                        yo = ysb.tile([P, 512], F32, tag="yo")
                        nc.vector.tensor_copy(out=yo, in_=yp)
                        eng = nc.sync if (tm + n) % 2 == 0 else nc.scalar
                        eng.dma_start(out=y[ms, ns], in_=yo)

    nc.compile()
    return nc


_NC_CACHE = None


def _get_nc():
    global _NC_CACHE
    if _NC_CACHE is None:
        _NC_CACHE = build_attention_nc()
    return _NC_CACHE


def kernel(x, w_q, w_vk, w_out, **run_kwargs):
    """Full inputs in, full output out. Shards over 8 NeuronCores."""
    b, n, dim = x.shape
    assert (b, n, dim) == (2, 2048, 1024)
    w_k = w_vk[:, :1024]
    w_v = w_vk[:, 1024:]

    in_maps = []
    for c in range(8):
        bi = c // 4
        hg = c % 4
        cs = slice(hg * E, (hg + 1) * E)
        in_maps.append({
            "x": np.ascontiguousarray(x[bi]).astype(np.float32),
            "wq": np.ascontiguousarray(w_q[:, cs]).astype(np.float32),
            "wk": np.ascontiguousarray(w_k[:, cs]).astype(np.float32),
            "wv": np.ascontiguousarray(w_v[:, cs]).astype(np.float32),
            "wo": np.ascontiguousarray(w_out[cs, :]).astype(np.float32),
        })

    nc = _get_nc()
    res = run_bass_kernel_spmd(nc, in_maps, core_ids=list(range(8)), **run_kwargs)
    out = np.zeros((2, 2048, 1024), dtype=np.float32)
    for c in range(8):
        out[c // 4] += res.results[c]["y"]
    if run_kwargs:
        kernel.last_results = res
    return out


# revision 34
# speedup vs baseline: 1.2128x; 1.0321x over previous
"""Trainium2 Bass kernel for nn_Attention_49907519980190.

Reference computation (b=2, n=2048, dim=1024, h=16, d=64):
    q = (x @ w_q)   -> (b, h, n, d)
    k, v = split(x @ w_vk)
    dots = (q @ k^T) * sqrt(d)          # NOTE: multiplies by 8
    attn = softmax(dots)
    out = (attn @ v) reassembled -> (b, n, h*d) @ w_out

Sharding (8 cores): batch x head-group parallel. Core c handles batch
b = c // 4 and heads 4*(c % 4) .. 4*(c % 4) + 4. Column-parallel
q/k/v projections, row-parallel out projection; the host sums the four
partial outputs per batch (the "all-reduce" of row-parallel TP).

Numerics: the softmax logits have std ~75 and the softmax is ~97%
one-hot, so Q/K/dots need fp32-class precision. The PE's in-matmul
accumulator is block-aligned (drops addends ~2^-11 below the running
max) but PSUM accumulation BETWEEN matmuls is exact fp32. So Q, K and
dots use a bf16 hi/lo decomposition: x = hi + lo with both bf16;
a@b ~= ah@bh + (al@bh + ah@bl), a main matmul plus one stacked
correction matmul ([qh;ql] x [kl;kh]) accumulating in PSUM
(measured ~5e-6 matmul rel err vs 2.4e-3 plain bf16).

Softmax: instead of an exact row max (expensive full-width DVE reduce
of PSUM), use a stride-2 subset max m^ and compute
u' = exp((l - 8*m^)/2) in fp32, then u = u'^2 on GPSIMD (exp-squared
doubles the usable logit-gap range to ~176; P(subset max short by
>176) ~ 1e-8 per row). The softmax denominator comes for free as a
65th all-ones column appended per-head to V (the PV matmul then
produces sum_j u_ij in column 64). Attention probabilities and the
whole post-softmax path are fp16.

Schedule: software-pipelined attention loop (PV for iteration N-3
issues between the dots of iteration N), PSUM S-quarters recycle
progressively, projections start as soon as the first token-group of
x^T lands, DMA-transposes all stay on the sync queue (issuing
transposes from two queues concurrently corrupts data - measured).
"""

import numpy as np

import concourse.bass as bass
import concourse.mybir as mybir
import concourse.tile as tile
from concourse import bacc
from concourse.bass_utils import run_bass_kernel_spmd
from concourse.masks import make_identity

F32 = mybir.dt.float32
BF16 = mybir.dt.bfloat16
FP16 = mybir.dt.float16
SUB = mybir.AluOpType.subtract
MULT = mybir.AluOpType.mult
MAX = mybir.AluOpType.max
AX = mybir.AxisListType.X
EXP = mybir.ActivationFunctionType.Exp

P = 128      # partitions
NTOK = 2048  # tokens per core (one batch slice)
DIM = 1024   # model dim
E = 256      # per-core projection width (4 heads x 64)
NH = 4       # heads per core
D = 64       # head dim
D1 = 65      # head dim + ones column (denominator trick)
KO = 8       # contraction chunks of 128 over DIM
TT = 16      # token tiles of 128
NG = 4       # token groups (of 512) for x^T
SCALE = 8.0  # sqrt(D); reference MULTIPLIES by it


def build_attention_nc():
    nc = bacc.Bacc("TRN2", target_bir_lowering=False, debug=False)

    x = nc.declare_dram_parameter("x", [NTOK, DIM], F32, isOutput=False)
    wq = nc.declare_dram_parameter("wq", [DIM, E], F32, isOutput=False)
    wk = nc.declare_dram_parameter("wk", [DIM, E], F32, isOutput=False)
    wv = nc.declare_dram_parameter("wv", [DIM, E], F32, isOutput=False)
    wo = nc.declare_dram_parameter("wo", [E, DIM], F32, isOutput=False)
    y = nc.declare_dram_parameter("y", [NTOK, DIM], F32, isOutput=True)

    with tile.TileContext(nc) as tc:
        with tc.tile_pool(name="persist", bufs=1) as persist:
            # Q^T stacked per head: rows 0:64 = q_hi, 64:128 = q_lo
            QTs = persist.tile([P, NH, NTOK], BF16)
            # K^T swapped-stacked: rows 0:64 = k_lo, 64:128 = k_hi
            KTsw = persist.tile([P, NH, NTOK], BF16)
            # K^T hi over zeros: main dots matmul shares the full 128-row
            # stationary [q_hi; q_lo] with the correction matmul (rows 64:128
            # multiply q_lo by zero)
            KThz = persist.tile([P, NH, NTOK], BF16)
            # V natural [tok_low, tok_hi, head*(64+ones)] fp16
            Vb = persist.tile([P, TT, NH * D1], FP16)
            Ob = persist.tile([P, TT, E], FP16)
            # O^T [emb_low, emb_hi(2), tok]
            OTb = persist.tile([P, 2, NTOK], FP16)
            wob = persist.tile([P, 2, DIM], FP16)
            ident = persist.tile([P, P], FP16)
            make_identity(nc, ident)
            identF = persist.tile([P, P], F32)
            make_identity(nc, identF)
            # ones columns of Vb (written once)
            vb4 = Vb.rearrange("p t (h c) -> p t h c", c=D1)
            nc.vector.memset(vb4[:, :, :, D:D1], 1.0)
            nc.vector.memset(KThz[64:128, :, :], 0.0)

            with tc.tile_pool(name="xpool", bufs=1) as xpool:
                # x^T hi/lo bf16 in 4 token-group tiles: [dim_low, dim_hi, 512]
                xh = []
                xl = []
                for g in range(NG):
                    xh_g = xpool.tile([P, KO, 512], BF16, tag=f"xh{g}")
                    xl_g = xpool.tile([P, KO, 512], BF16, tag=f"xl{g}")
                    xh.append(xh_g)
                    xl.append(xl_g)
                wqh = xpool.tile([P, KO, E], BF16)
                wql = xpool.tile([P, KO, E], BF16)
                wkh = xpool.tile([P, KO, E], BF16)
                wkl = xpool.tile([P, KO, E], BF16)
                wvh = xpool.tile([P, KO, E], BF16)

                # ---------- Phase A/B: weights, x split hi/lo + transpose
                with (
                    tc.tile_pool(name="psT", bufs=3, space="PSUM") as psT,
                    tc.tile_pool(name="psA", bufs=2, space="PSUM") as psA,
                    tc.tile_pool(name="stage", bufs=3) as stage,
                ):
                    for wsrc, hdst, ldst in ((wk, wkh, wkl), (wq, wqh, wql)):
                        wf = stage.tile([P, KO, E], F32, tag="wf", bufs=1)
                        nc.sync.dma_start(
                            out=wf,
                            in_=wsrc[:, :].rearrange("(ko p) e -> p ko e", p=P),
                        )
                        nc.vector.tensor_copy(out=hdst, in_=wf)
                        nc.vector.tensor_tensor(out=ldst, in0=wf, in1=hdst, op=SUB)
                    wf = stage.tile([P, KO, E], F32, tag="wf", bufs=1)
                    nc.sync.dma_start(
                        out=wf, in_=wv[:, :].rearrange("(ko p) e -> p ko e", p=P)
                    )
                    nc.scalar.copy(out=wvh, in_=wf)
                    wof = stage.tile([P, 2, DIM], F32, tag="wof", bufs=1)
                    nc.sync.dma_start(
                        out=wof, in_=wo[:, :].rearrange("(eo p) d -> p eo d", p=P)
                    )
                    nc.scalar.copy(out=wob, in_=wof)

                    for tt in range(TT):
                        g, lt = tt // 4, tt % 4
                        ts = slice(tt * P, (tt + 1) * P)
                        gs = slice(lt * P, (lt + 1) * P)
                        xf = stage.tile([P, DIM], F32, tag="xf")
                        nc.gpsimd.dma_start(out=xf, in_=x[ts, :])
                        for half in range(2):
                            cr = slice(half * 4, half * 4 + 4)
                            pt4 = psT.tile([P, 4, P], F32, tag="pt4")
                            for c4 in range(4):
                                c = half * 4 + c4
                                nc.tensor.transpose(
                                    pt4[:, c4, :], xf[:, c * P:(c + 1) * P],
                                    identF[:, :],
                                )
                            nc.vector.tensor_copy(
                                out=xh[g][:, cr, gs], in_=pt4
                            )
                            nc.vector.tensor_tensor(
                                out=xl[g][:, cr, gs], in0=pt4,
                                in1=xh[g][:, cr, gs], op=SUB,
                            )

                # ---------- Phase C: projections K^T, Q^T (hi/lo), V (plain)
                if True:
                    def proj_hilo(wh, wl, m, g):
                        pr = psA.tile([P, 512], F32, tag="pr")
                        ms = slice(m * P, (m + 1) * P)
                        for c in range(KO):
                            nc.tensor.matmul(
                                pr[:, :], wh[:, c, ms], xh[g][:, c, :],
                                start=(c == 0), stop=False,
                            )
                        for c in range(KO):
                            nc.tensor.matmul(
                                pr[:, :], wl[:, c, ms], xh[g][:, c, :],
                                start=False, stop=False,
                            )
                        for c in range(KO):
                            nc.tensor.matmul(
                                pr[:, :], wh[:, c, ms], xl[g][:, c, :],
                                start=False, stop=(c == KO - 1),
                            )
                        return pr

                    for g in range(NG):
                        ns = slice(g * 512, (g + 1) * 512)
                        for m in range(2):
                            pr = proj_hilo(wkh, wkl, m, g)
                            for hh in range(2):
                                h = 2 * m + hh
                                rows = slice(hh * 64, hh * 64 + 64)
                                nc.scalar.copy(
                                    out=KThz[0:64, h, ns], in_=pr[rows, :]
                                )
                                nc.scalar.copy(
                                    out=KTsw[64:128, h, ns], in_=pr[rows, :]
                                )
                                nc.vector.tensor_tensor(
                                    out=KTsw[0:64, h, ns], in0=pr[rows, :],
                                    in1=KThz[0:64, h, ns], op=SUB,
                                )
                        for m in range(2):
                            pr = proj_hilo(wqh, wql, m, g)
                            for hh in range(2):
                                h = 2 * m + hh
                                rows = slice(hh * 64, hh * 64 + 64)
                                nc.scalar.copy(
                                    out=QTs[0:64, h, ns], in_=pr[rows, :]
                                )
                                nc.vector.tensor_tensor(
                                    out=QTs[64:128, h, ns], in0=pr[rows, :],
                                    in1=QTs[0:64, h, ns], op=SUB,
                                )
                        for lt in range(4):
                            tm = g * 4 + lt
                            pr = psA.tile([P, 512], F32, tag="pr")
                            for c in range(KO):
                                nc.tensor.matmul(
                                    pr[:, :E], xh[g][:, c, lt * P:(lt + 1) * P],
                                    wvh[:, c, :],
                                    start=(c == 0), stop=(c == KO - 1),
                                )
                            for h in range(NH):
                                nc.scalar.copy(
                                    out=Vb[:, tm, h * D1:h * D1 + D],
                                    in_=pr[:, h * D:(h + 1) * D],
                                )

            # ---------- Phase D: attention, software-pipelined depth 3
            with (
                tc.tile_pool(name="psS", bufs=7, space="PSUM") as psS,
                tc.tile_pool(name="psO", bufs=1, space="PSUM") as psO,
                tc.tile_pool(name="attn_sb", bufs=3) as attn_sb,
                tc.tile_pool(name="attn_small", bufs=6) as attn_small,
            ):
                pending = []

                def issue_dots(h, it):
                    isl = slice(it * P, (it + 1) * P)
                    quarters = []
                    for nn in range(4):
                        Sq = psS.tile([P, 512], F32, tag="S")
                        quarters.append(Sq)
                    # all 8 matmuls share one stationary [q_hi; q_lo]
                    for nn in range(4):
                        ns = slice(nn * 512, (nn + 1) * 512)
                        nc.tensor.matmul(
                            quarters[nn][:, :], QTs[:, h, isl], KThz[:, h, ns],
                            start=True, stop=False,
                        )
                    for nn in range(4):
                        ns = slice(nn * 512, (nn + 1) * 512)
                        nc.tensor.matmul(
                            quarters[nn][:, :], QTs[:, h, isl], KTsw[:, h, ns],
                            start=False, stop=True,
                        )
                    return quarters

                def issue_softmax(h, it, quarters):
                    mx4 = attn_small.tile([P, 4], F32, tag="mx4")
                    for nn in range(4):
                        nc.vector.tensor_reduce(
                            out=mx4[:, nn:nn + 1], in_=quarters[nn], axis=AX, op=MAX
                        )
                    nmx = attn_small.tile([P, 1], F32, tag="nmx")
                    nc.vector.tensor_reduce(
                        out=nmx, in_=mx4, axis=AX, op=MAX, negate=True
                    )
                    bias8 = attn_small.tile([P, 1], F32, tag="bias8")
                    nc.gpsimd.tensor_scalar_mul(bias8, nmx, SCALE)
                    u = attn_sb.tile([P, NTOK], FP16, tag="u", bufs=5)
                    for nn in range(4):
                        cs = slice(nn * 512, (nn + 1) * 512)
                        nc.scalar.activation(
                            out=u[:, cs], in_=quarters[nn], func=EXP,
                            bias=bias8, scale=SCALE,
                        )
                    PT = attn_sb.tile([P, TT, P], FP16, tag="PT", bufs=8)
                    nc.sync.dma_start_transpose(out=PT, in_=u)
                    return PT

                def issue_pv(h, it, PT):
                    O_ps = psO.tile([P, D1], F32, tag="O")
                    for jo in range(TT):
                        nc.tensor.matmul(
                            O_ps[:, :], PT[:, jo, :],
                            Vb[:, jo, h * D1:(h + 1) * D1],
                            start=(jo == 0), stop=(jo == TT - 1),
                        )
                    rec = attn_small.tile([P, 1], F32, tag="rec")
                    nc.vector.reciprocal(out=rec, in_=O_ps[:, D:D1])
                    nc.scalar.activation(
                        out=Ob[:, it, h * D:(h + 1) * D], in_=O_ps[:, :D],
                        func=mybir.ActivationFunctionType.Copy, scale=rec,
                    )

                step = 0
                for h in range(NH):
                    for it in range(TT):
                        quarters = issue_dots(h, it)
                        # batch PV issue in pairs every other iteration so the
                        # dots blocks form long dense PE-array stretches (HAM)
                        if step % 2 == 1:
                            while len(pending) > 4:
                                issue_pv(*pending.pop(0))
                        PT = issue_softmax(h, it, quarters)
                        pending.append((h, it, PT))
                        step += 1
                while pending:
                    issue_pv(*pending.pop(0))

            # ---------- Phase E: O^T then y = O @ wo
            with (
                tc.tile_pool(name="psE", bufs=2, space="PSUM") as psE,
                tc.tile_pool(name="ysb", bufs=3) as ysb,
            ):
                for tt in range(TT):
                    ts = slice(tt * P, (tt + 1) * P)
                    for eo in range(2):
                        pt = psE.tile([P, P], FP16, tag="pt")
                        nc.tensor.transpose(
                            pt[:, :], Ob[:, tt, eo * P:(eo + 1) * P], ident[:, :]
                        )
                        nc.vector.tensor_copy(out=OTb[:, eo, ts], in_=pt)
                for tm in range(TT):
                    ms = slice(tm * P, (tm + 1) * P)
                    for n in range(2):
                        ns = slice(n * 512, (n + 1) * 512)
                        yp = psE.tile([P, 512], F32, tag="yp")
                        for eo in range(2):
                            nc.tensor.matmul(
                                yp[:, :], OTb[:, eo, ms], wob[:, eo, ns],
                                start=(eo == 0), stop=(eo == 1),
                            )
                        yo = ysb.tile([P, 512], F32, tag="yo")
                        nc.vector.tensor_copy(out=yo, in_=yp)
                        eng = nc.sync if (tm + n) % 2 == 0 else nc.scalar
                        eng.dma_start(out=y[ms, ns], in_=yo)

    nc.compile()
    return nc


_NC_CACHE = None


def _get_nc():
    global _NC_CACHE
    if _NC_CACHE is None:
        _NC_CACHE = build_attention_nc()
    return _NC_CACHE


def kernel(x, w_q, w_vk, w_out, **run_kwargs):
    """Full inputs in, full output out. Shards over 8 NeuronCores."""
    b, n, dim = x.shape
    assert (b, n, dim) == (2, 2048, 1024)
    w_k = w_vk[:, :1024]
    w_v = w_vk[:, 1024:]

    in_maps = []
    for c in range(8):
        bi = c // 4
        hg = c % 4
        cs = slice(hg * E, (hg + 1) * E)
        in_maps.append({
            "x": np.ascontiguousarray(x[bi]).astype(np.float32),
            "wq": np.ascontiguousarray(w_q[:, cs]).astype(np.float32),
            "wk": np.ascontiguousarray(w_k[:, cs]).astype(np.float32),
            "wv": np.ascontiguousarray(w_v[:, cs]).astype(np.float32),
            "wo": np.ascontiguousarray(w_out[cs, :]).astype(np.float32),
        })

    nc = _get_nc()
    res = run_bass_kernel_spmd(nc, in_maps, core_ids=list(range(8)), **run_kwargs)
    out = np.zeros((2, 2048, 1024), dtype=np.float32)
    for c in range(8):
        out[c // 4] += res.results[c]["y"]
    if run_kwargs:
        kernel.last_results = res
    return out
